# revision 48
# baseline (speedup 1.0000x reference)
"""Trainium2 Bass kernel for nn_AttentionBlock (B=8, L=2048, E=512, FF=2048).

Strategy: data-parallel over batch — core b computes batch item b end-to-end
(no collectives). All activations live transposed ([feature, token], feature on
partitions) so every matmul contracts over the partition dim with natural
layouts and no on-device transposes.

Key optimizations over the fp32r baseline:
  * M-fusion: scores = x @ (SCALE Wq^T Wk) @ x^T, so the k-projection (and its
    evictions + SBUF residency) disappears; scores take x8 directly as the
    stationary operand.
  * fp8 DoubleRow matmuls (contract 256 per instruction, 2x PE throughput) for
    the t-projection, v-projection, scores, softmax column-sum and AV. Small-
    sigma operands (M, Wv) are pre-scaled by powers of two into e4m3's normal
    range; the compensation folds into existing ACT scale knobs for free.
    pexp uses e5m2 for range (no row-max subtraction), everything else e4m3.
  * exp carries a 2^-5 bias (cancels in the softmax ratio) so e5m2 can't
    overflow.
  * FFN1 and FFN2 also run fp8 DoubleRow (weights shipped pre-scaled e4m3
    from the host); the last l-chunk's FFN2/residual/LN2 pipeline is split
    into column halves so the final LN2 chain hides under matmuls.
  * epilogue elementwise stays on ACT + DVE only: GpSimd elementwise is
    3-10x slower and poisons the shared-stat-tile WAR chains; it only issues
    DMAs. Early DMAs go on the sync+gpsimd rings in first-need order (each
    dma_start costs ~700ns of issuing-engine time).

Measured: 217-221us (fast device allocation) / ~260us (slow allocation,
+43ns/PE-dispatch lottery outside kernel control), rel err 1.72e-2 vs the
2e-2 gate, bit-stable across runs. FFN1_FP8=False trades ~10us for rel err
1.38e-2 if more margin is ever needed.
"""
import math
from contextlib import ExitStack

import ml_dtypes
import numpy as np

import concourse.bass as bass
import concourse.bacc as bacc
import concourse.tile as tile
from concourse import mybir
from concourse.bass_utils import run_bass_kernel_spmd

P = 128
B, L, E, FF = 8, 2048, 512, 2048
NDOM = 32
EPS = 1e-5
SCALE = (1.0 / math.sqrt(E)) * 2.0 * math.log(NDOM)
EXPB = -5.0 * math.log(2.0)   # exp(s+EXPB)=exp(s)/32, cancels in softmax

EO = E // P           # 4  e-chunks
FO = FF // P          # 16 f-chunks
LC = 512              # l-chunk (matmul free dim)
NLC = L // LC         # 4  l-chunks
SB = L // P           # 16 s-blocks
NPR = EO // 2         # 2  DoubleRow eo-pairs

F32 = mybir.dt.float32
F32R = mybir.dt.float32r
F16 = mybir.dt.float16
F8E4 = mybir.dt.float8e4
F8E5 = mybir.dt.float8e5
AF = mybir.ActivationFunctionType
OP = mybir.AluOpType
DR = mybir.MatmulPerfMode.DoubleRow

M_SC = 32.0           # M8 = 32*M (e4m3 normal range)
T_SC = 8.0            # t8 = 8*t  -> exp scale 1/8
WV_SC = 16.0          # wv8 = 16*Wv^T -> vt8 evict scale 1/16

FFN1_FP8 = True       # fp8 DoubleRow FFN1: 2x PE speed, ~1e-2 rel err
FFN2_FP8 = True

_TRACE = False
LAST_RESULT = None
_CACHE = {}


def _round_fp32r(x):
    """Round-to-nearest-even fp32 -> fp32r (low 12 mantissa bits cleared)."""
    u = np.ascontiguousarray(x, dtype=np.float32).view(np.uint32)
    frac = u & np.uint32(0xFFF)
    base = u & np.uint32(0xFFFFF000)
    up = (frac > 0x800) | ((frac == 0x800) & (((u >> 12) & 1) == 1))
    return (base + np.where(up, np.uint32(0x1000), np.uint32(0))).view(np.float32)


def _build(ln1_trivial, ln2_trivial, b2_zero):
    nc = bacc.Bacc("TRN2", debug=False, target_bir_lowering=False, num_devices=B)

    xt_d = nc.dram_tensor("xt", [E, L], F32R, kind="ExternalInput")
    x8_d = nc.dram_tensor("x8", [E, L], F8E4, kind="ExternalInput")
    m8_d = nc.dram_tensor("m8", [E, E], F8E4, kind="ExternalInput")
    wv8_d = nc.dram_tensor("wv8", [E, E], F8E4, kind="ExternalInput")
    w1t_d = nc.dram_tensor("w1t", [E, FF], F8E4 if FFN1_FP8 else F16,
                           kind="ExternalInput")
    w2t_d = nc.dram_tensor("w2t", [FF, E], F8E4 if FFN2_FP8 else F16,
                           kind="ExternalInput")
    b1_d = nc.dram_tensor("b1v", [FF], F32, kind="ExternalInput")
    b2_d = None if b2_zero else nc.dram_tensor("b2v", [E], F32, kind="ExternalInput")
    ln1w_d = ln1b_d = ln2w_d = ln2b_d = None
    if not ln1_trivial:
        ln1w_d = nc.dram_tensor("ln1w", [E], F32, kind="ExternalInput")
        ln1b_d = nc.dram_tensor("ln1b", [E], F32, kind="ExternalInput")
    if not ln2_trivial:
        ln2w_d = nc.dram_tensor("ln2w", [E], F32, kind="ExternalInput")
        ln2b_d = nc.dram_tensor("ln2b", [E], F32, kind="ExternalInput")
    out_d = nc.dram_tensor("outt", [E, L], F32, kind="ExternalOutput")

    xt_r = xt_d.ap().rearrange("(eo p) l -> p eo l", p=P)
    x8_r = x8_d.ap().rearrange("(eo p) l -> p eo l", p=P)
    m8_r = m8_d.ap().rearrange("(eo p) f -> p eo f", p=P)
    wv8_r = wv8_d.ap().rearrange("(eo p) f -> p eo f", p=P)
    w1t_r = w1t_d.ap().rearrange("(eo p) f -> p eo f", p=P)
    w2t_r = w2t_d.ap().rearrange("(fo p) e -> p fo e", p=P)
    out_r = out_d.ap().rearrange("(eo p) l -> p eo l", p=P)

    with tile.TileContext(nc) as tc, ExitStack() as stk:
        const = stk.enter_context(tc.tile_pool(name="const", bufs=1))
        px = stk.enter_context(tc.tile_pool(name="px", bufs=1))
        px8 = stk.enter_context(tc.tile_pool(name="px8", bufs=1))
        pstat = stk.enter_context(tc.tile_pool(name="pstat", bufs=1))
        ph = stk.enter_context(tc.tile_pool(name="ph", bufs=2))
        pysq = stk.enter_context(tc.tile_pool(name="pysq", bufs=1))
        pwearly = stk.enter_context(tc.tile_pool(name="pwearly", bufs=1))

        ones_r = const.tile([P, P], F32R)
        ones_f2 = const.tile([P, 2, P], F32)
        ones8 = const.tile([P, 2, P], F8E4)
        eps_t = const.tile([P, 1], F32)
        expb_t = const.tile([P, 1], F32)
        b1_t = const.tile([P, FO], F32)
        nc.vector.memset(ones_r[:].bitcast(F32), 1.0)
        nc.vector.memset(ones_f2[:], 1.0)
        nc.vector.tensor_copy(ones8[:], ones_f2[:])
        nc.vector.memset(eps_t[:], EPS)
        nc.vector.memset(expb_t[:], EXPB)
        warm = const.tile([P, 8], F32)
        nc.vector.memset(warm[:], 0.0)
        # touch Exp so the ACT table load hides under DMA-queue startup
        nc.scalar.activation(warm[:], warm[:], AF.Exp, bias=expb_t[:])
        b1_r = b1_d.ap().rearrange("(fo p) -> p fo", p=P)
        b2_t = None
        if b2_d is not None:
            b2_t = const.tile([P, EO], F32)
            nc.sync.dma_start(b2_t[:], b2_d.ap().rearrange("(eo p) -> p eo", p=P))
        ln1w_t = ln1b_t = ln2w_t = ln2b_t = None
        if ln1w_d is not None:
            ln1w_t = const.tile([P, EO], F32)
            ln1b_t = const.tile([P, EO], F32)
            nc.sync.dma_start(ln1w_t[:], ln1w_d.ap().rearrange("(eo p) -> p eo", p=P))
            nc.sync.dma_start(ln1b_t[:], ln1b_d.ap().rearrange("(eo p) -> p eo", p=P))
        if ln2w_d is not None:
            ln2w_t = const.tile([P, EO], F32)
            ln2b_t = const.tile([P, EO], F32)
            nc.sync.dma_start(ln2w_t[:], ln2w_d.ap().rearrange("(eo p) -> p eo", p=P))
            nc.sync.dma_start(ln2b_t[:], ln2b_d.ap().rearrange("(eo p) -> p eo", p=P))

        xt = px.tile([P, EO, L], F32R)          # x^T, becomes y = x + attn
        x8 = px8.tile([P, EO, L], F8E4)         # e4m3 copy for DR matmuls
        state = {}

        def ln_stats_rest(i, tag, s_ps, s2_ps, y_sl):
            """negmean/msq on ACT, var via fused DVE op, rstd via sqrt+recip."""
            w = s_ps.shape[-1]
            negmean = pstat.tile([P, LC], F32, tag="nm", name=f"nm{tag}_{i}")
            msq = pstat.tile([P, LC], F32, tag="msq", name=f"msq{tag}_{i}")
            ex2 = pstat.tile([P, LC], F32, tag="ex2", name=f"ex2{tag}_{i}")
            rstd = pstat.tile([P, LC], F32, tag="rstd", name=f"rstd{tag}_{i}")
            nc.scalar.activation(negmean[:, 0:w], s_ps[:], AF.Copy,
                                 scale=-1.0 / E)
            nc.scalar.activation(msq[:, 0:w], s_ps[:], AF.Square, scale=1.0 / E)
            nc.vector.scalar_tensor_tensor(ex2[:, 0:w], s2_ps[:], 1.0 / E,
                                           msq[:, 0:w], OP.mult, OP.subtract)
            nc.scalar.activation(ex2[:, 0:w], ex2[:, 0:w], AF.Sqrt,
                                 bias=eps_t[:])
            nc.vector.reciprocal_approx_fast(rstd[:, 0:w], ex2[:, 0:w])
            return y_sl, negmean[:, 0:w], rstd[:, 0:w]

        def ln1_apply(i):
            y_sl, negmean, rstd = state.pop(("ln1", i))
            h = ph.tile([P, EO, LC], F16, tag="h", name=f"h{i}")
            h8 = None
            if FFN1_FP8:
                h8 = ph.tile([P, EO, LC], F8E4, tag="h8", name=f"h8_{i}")
            for ec in range(EO):
                t = pstat.tile([P, LC], F32, tag="lnapp", name=f"la1_{i}_{ec}")
                nc.vector.tensor_tensor(t[:], y_sl[ec].bitcast(F32),
                                        negmean[:], OP.add)
                if ln1_trivial:
                    nc.vector.tensor_tensor(h[:, ec, :], t[:], rstd[:], OP.mult)
                else:
                    nc.vector.tensor_tensor(t[:], t[:], rstd[:], OP.mult)
                    nc.scalar.activation(h[:, ec, :], t[:], AF.Identity,
                                         bias=ln1b_t[:, ec:ec + 1],
                                         scale=ln1w_t[:, ec:ec + 1])
                if h8 is not None:
                    nc.vector.tensor_tensor(h8[:, ec, :], t[:], rstd[:],
                                            OP.mult)
            state[("h", i)] = (h, h8)

        with tc.tile_pool(name="pkv", bufs=1) as pkv, \
             tc.tile_pool(name="pm8", bufs=1) as pm8, \
             tc.tile_pool(name="psMM", bufs=2, space="PSUM") as psMM:
            m8 = pm8.tile([P, EO, E], F8E4)     # 32*M
            vt8 = pkv.tile([P, SB, E], F8E4)    # v natural [s, e]

            # ------------- phase A: loads (all fp8 operands from host) ------
            with tc.tile_pool(name="pwv", bufs=1) as pwv:
                wv8 = pwv.tile([P, EO, E], F8E4)

                # PE warm-up: ~4.3us of const matmuls fill the DMA-startup
                # head gap and ramp the PE p-state to 2.4GHz before real work
                warm_mv = pwv.tile([P, LC], F32R)
                nc.vector.memset(warm_mv[:].bitcast(F32), 1.0)
                warm_ps = psMM.tile([P, LC], F32, tag="mm", name="warmps")
                warm_sb = pwv.tile([P, 8], F32)
                for k in range(20):
                    nc.tensor.matmul(warm_ps[:], ones_r[:], warm_mv[:],
                                     start=(k == 0), stop=(k == 19))
                nc.scalar.activation(warm_sb[:], warm_ps[:, 0:8], AF.Copy)

                def load_x(lc, engs=(nc.scalar, nc.gpsimd)):
                    ls = lc * LC
                    for eo in range(EO):
                        engs[eo % len(engs)].dma_start(
                            xt[:, eo, ls:ls + LC], xt_r[:, eo, ls:ls + LC])

                # DMA plan: each dma_start costs ~700ns of issuing-engine
                # sequencer time, so early loads go only on the sync + gpsimd
                # rings (both compute-idle), first-need first: m8/wv8 lead on
                # separate rings, then x8 lc0-slices, then the x8 remainder
                # and xt as four big transfers. The scalar ring carries only
                # late FFN weights so ACT's real work isn't queued behind
                # enqueues.
                nc.sync.dma_start(m8[:, 0:2, :], m8_r[:, 0:2, :])
                nc.gpsimd.dma_start(m8[:, 2:4, :], m8_r[:, 2:4, :])
                for eo in range(EO):
                    (nc.sync, nc.gpsimd)[eo % 2].dma_start(
                        x8[:, eo, 0:LC], x8_r[:, eo, 0:LC])
                nc.gpsimd.dma_start(wv8[:, 0:2, :], wv8_r[:, 0:2, :])
                nc.sync.dma_start(wv8[:, 2:4, :], wv8_r[:, 2:4, :])
                for eo in range(EO):
                    (nc.sync, nc.gpsimd)[eo % 2].dma_start(
                        x8[:, eo, LC:], x8_r[:, eo, LC:])
                for eo in range(EO):
                    (nc.sync, nc.gpsimd)[eo % 2].dma_start(
                        xt[:, eo, :], xt_r[:, eo, :])

                # ------------- attention -------------
                with (
                    tc.tile_pool(name="pq", bufs=2) as pq,
                    tc.tile_pool(name="pp", bufs=2) as pp,
                    tc.tile_pool(name="paon", bufs=1) as paon,
                    tc.tile_pool(name="psAO", bufs=4, space="PSUM") as psAO,
                    tc.tile_pool(name="psCS", bufs=1, space="PSUM") as psCS,
                    tc.tile_pool(name="psSB", bufs=1, space="PSUM") as psSB,
                ):
                    # first FFN weight slices land here during attention
                    w1e = pwearly.tile([P, EO, 4 * P],
                                       F8E4 if FFN1_FP8 else F16)
                    w2e = pwearly.tile([P, 2, E], F8E4 if FFN2_FP8 else F16)
                    for fo in range(4):
                        nc.scalar.dma_start(w1e[:, :, fo * P:(fo + 1) * P],
                                            w1t_r[:, :, fo * P:(fo + 1) * P])
                        if fo < 2:
                            nc.gpsimd.dma_start(w2e[:, fo, :], w2t_r[:, fo, :])

                    def t_proj_part(t8, lc, eb, act_evict):
                        ls = lc * LC
                        tp = psMM.tile([P, LC], F32, tag="mm",
                                       name=f"tp{lc}_{eb}")
                        for pr in range(NPR):
                            nc.tensor.matmul(
                                tp[:], m8[:, 2 * pr:2 * pr + 2,
                                          eb * P:(eb + 1) * P],
                                x8[:, 2 * pr:2 * pr + 2, ls:ls + LC],
                                start=(pr == 0), stop=(pr == NPR - 1),
                                perf_mode=DR)
                        if act_evict:
                            nc.scalar.activation(t8[:, eb, :], tp[:], AF.Copy,
                                                 scale=T_SC / M_SC)
                        else:
                            nc.vector.tensor_scalar_mul(t8[:, eb, :], tp[:],
                                                        T_SC / M_SC)

                    def t_proj(lc):
                        t8 = pq.tile([P, EO, LC], F8E4, tag="q", name=f"t8_{lc}")
                        for eb in range(EO):
                            t_proj_part(t8, lc, eb, eb % 2 == 1)
                        return t8

                    def v_proj(sb):
                        vp = psMM.tile([P, E], F32, tag="mm", name=f"vp{sb}")
                        for pr in range(NPR):
                            nc.tensor.matmul(
                                vp[:], x8[:, 2 * pr:2 * pr + 2,
                                          sb * P:(sb + 1) * P],
                                wv8[:, 2 * pr:2 * pr + 2, :],
                                start=(pr == 0), stop=(pr == NPR - 1),
                                perf_mode=DR)
                        # GPSIMD can't read PSUM: evict on ACT/DVE alternately
                        if sb % 2 == 0:
                            nc.scalar.activation(vt8[:, sb, :], vp[:], AF.Copy,
                                                 scale=1.0 / WV_SC)
                        else:
                            nc.vector.tensor_scalar_mul(vt8[:, sb, :], vp[:],
                                                        1.0 / WV_SC)

                    # chunk-0 LN1 pieces, injected into chunk 1's stream
                    def ln1c0_sq():
                        y_sl = [xt[:, ec, 0:LC] for ec in range(EO)]
                        ysq = pysq.tile([P, EO, LC], F32R, tag="ysq",
                                        name="ysq1_0")
                        for ec in range(EO):
                            nc.scalar.activation(ysq[:, ec, :],
                                                 y_sl[ec].bitcast(F32),
                                                 AF.Square)
                        state["c0"] = (y_sl, ysq)

                    def ln1c0_sum1():
                        y_sl, ysq = state["c0"]
                        s_ps = psSB.tile([P, LC], F32, tag="sums", name="s1_0")
                        for ec in range(EO):
                            nc.tensor.matmul(s_ps[:], ones_r[:], y_sl[ec],
                                             start=(ec == 0), stop=(ec == EO - 1))
                        negmean = pstat.tile([P, LC], F32, tag="nm", name="nm1_0")
                        nc.scalar.activation(negmean[:], s_ps[:], AF.Copy,
                                             scale=-1.0 / E)
                        msq = pstat.tile([P, LC], F32, tag="msq", name="msq1_0")
                        nc.scalar.activation(msq[:], s_ps[:], AF.Square,
                                             scale=1.0 / E)
                        state["c0b"] = (negmean, msq)

                    def ln1c0_sum2():
                        y_sl, ysq = state.pop("c0")
                        negmean, msq = state.pop("c0b")
                        s2_ps = psSB.tile([P, LC], F32, tag="sums", name="s2_0")
                        for ec in range(EO):
                            nc.tensor.matmul(s2_ps[:], ones_r[:], ysq[:, ec, :],
                                             start=(ec == 0), stop=(ec == EO - 1))
                        ex2 = pstat.tile([P, LC], F32, tag="ex2", name="ex21_0")
                        rstd = pstat.tile([P, LC], F32, tag="rstd", name="rstd1_0")
                        nc.vector.scalar_tensor_tensor(ex2[:], s2_ps[:], 1.0 / E,
                                                       msq[:], OP.mult, OP.subtract)
                        nc.scalar.activation(ex2[:], ex2[:], AF.Sqrt, bias=eps_t[:])
                        nc.vector.reciprocal_approx_fast(rstd[:], ex2[:])
                        state[("ln1", 0)] = ([xt[:, ec, 0:LC] for ec in range(EO)],
                                             negmean, rstd)

                    t8s = {0: t_proj(0)}
                    for lc in range(NLC):
                        ls = lc * LC
                        t8 = t8s.pop(lc)
                        pexp = pp.tile([P, SB, LC], F8E5, tag="pexp",
                                       name=f"pexp{lc}")
                        ao = [psAO.tile([P, LC], F32, tag="ao",
                                        name=f"ao{lc}_{e}") for e in range(EO)]
                        cs = psCS.tile([P, LC], F32, tag="cs", name=f"cs{lc}")

                        inject = {}
                        if lc == 1:
                            inject = {4: ln1c0_sq, 7: ln1c0_sum1, 10: ln1c0_sum2,
                                      13: lambda: ln1_apply(0)}

                        st_ps = []

                        def scores(sb, t8=t8, st_ps=st_ps, lc=lc):
                            sp = psMM.tile([P, LC], F32, tag="mm",
                                           name=f"sp{lc}_{sb}")
                            for pr in range(NPR):
                                nc.tensor.matmul(
                                    sp[:], x8[:, 2 * pr:2 * pr + 2,
                                              sb * P:(sb + 1) * P],
                                    t8[:, 2 * pr:2 * pr + 2, :],
                                    start=(pr == 0), stop=(pr == NPR - 1),
                                    perf_mode=DR)
                            st_ps.append(sp)

                        def expevict(sb, pexp=pexp, st_ps=st_ps):
                            nc.scalar.activation(pexp[:, sb, :], st_ps[sb][:],
                                                 AF.Exp, bias=expb_t[:],
                                                 scale=1.0 / T_SC)

                        def av_pair(j, pexp=pexp, ao=ao, cs=cs):
                            nc.tensor.matmul(cs[:], ones8[:],
                                             pexp[:, 2 * j:2 * j + 2, :],
                                             start=(j == 0), stop=(j == SB // 2 - 1),
                                             perf_mode=DR)
                            for eb in range(EO):
                                nc.tensor.matmul(
                                    ao[eb][:],
                                    vt8[:, 2 * j:2 * j + 2, eb * P:(eb + 1) * P],
                                    pexp[:, 2 * j:2 * j + 2, :],
                                    start=(j == 0), stop=(j == SB // 2 - 1),
                                    perf_mode=DR)

                        # next chunk's t-projection is interleaved at sb
                        # 13/14 so its evictions land before the boundary;
                        # during the lc==1 injections DVE is busy with
                        # ln1_apply, so evict on ACT there instead
                        for sb in range(SB):
                            if lc == 0:
                                v_proj(sb)
                            scores(sb)
                            expevict(sb)
                            if sb % 2 == 1 and sb > 1:
                                av_pair(sb // 2 - 1)
                            if sb in inject:
                                inject[sb]()
                            if lc + 1 < NLC and sb in (13, 14):
                                if sb == 13:
                                    t8n = pq.tile([P, EO, LC], F8E4, tag="q",
                                                  name=f"t8_{lc + 1}")
                                    t8s[lc + 1] = t8n
                                for eb in (0, 1) if sb == 13 else (2, 3):
                                    t_proj_part(t8s[lc + 1], lc + 1, eb,
                                                lc == 1 or eb % 2 == 1)
                        av_pair(SB // 2 - 1)

                        rcs = pstat.tile([P, LC], F32, tag="rcs")
                        nc.vector.reciprocal_approx_fast(rcs[:], cs[:])
                        # y = x + ao * rcs (in place into xt), all on DVE —
                        # GPSIMD elementwise is 3-10x slower on this HW
                        for ec in range(EO):
                            aon = paon.tile([P, LC], F32, tag="aon",
                                            name=f"aon{lc}_{ec}")
                            nc.vector.tensor_tensor(aon[:], ao[ec][:], rcs[:],
                                                    OP.mult)
                            nc.vector.tensor_tensor(
                                xt[:, ec, ls:ls + LC],
                                xt[:, ec, ls:ls + LC].bitcast(F32), aon[:],
                                OP.add)

                    state["wearly"] = (w1e, w2e)

        # ---------------- phase C: LN1, FFN, LN2 per l-chunk ----------------
        with (
            tc.tile_pool(name="pw1", bufs=1) as pw1,
            tc.tile_pool(name="pw2", bufs=1) as pw2,
            tc.tile_pool(name="py2", bufs=1) as py2,
            tc.tile_pool(name="prelu", bufs=1) as prelu,
            tc.tile_pool(name="pout", bufs=1) as pout,
            tc.tile_pool(name="psF1", bufs=2, space="PSUM") as psF1,
            tc.tile_pool(name="psF2", bufs=4, space="PSUM") as psF2,
            tc.tile_pool(name="psS", bufs=2, space="PSUM") as psS,
        ):
            w1t = pw1.tile([P, EO, FF], F8E4 if FFN1_FP8 else F16)
            w2t = pw2.tile([P, FO, E], F8E4 if FFN2_FP8 else F16)
            nc.scalar.dma_start(b1_t[:], b1_r)
            w1e, w2e = state.pop("wearly")
            # sliced loads so FFN matmuls start as soon as each slice lands
            for g in range(1, EO):
                nc.sync.dma_start(w1t[:, :, g * 4 * P:(g + 1) * 4 * P],
                                  w1t_r[:, :, g * 4 * P:(g + 1) * 4 * P])
            for fo in range(2, FO):
                nc.gpsimd.dma_start(w2t[:, fo, :], w2t_r[:, fo, :])

            def w1_sl(fo):
                if fo < 4:
                    return w1e[:, :, fo * P:(fo + 1) * P]
                return w1t[:, :, fo * P:(fo + 1) * P]

            def w2_sl(fo):
                if fo < 2:
                    return w2e[:, fo, :]
                return w2t[:, fo, :]

            def ln_pre(i):
                """squares + partition sums + rstd for chunk i's LN1."""
                ls = i * LC
                y_sl = [xt[:, ec, ls:ls + LC] for ec in range(EO)]
                ysq = pysq.tile([P, EO, LC], F32R, tag="ysq", name=f"ysq1_{i}")
                for ec in range(EO):
                    nc.vector.tensor_tensor(ysq[:, ec, :], y_sl[ec].bitcast(F32),
                                            y_sl[ec].bitcast(F32), OP.mult)
                s_ps = psS.tile([P, LC], F32, tag="sums", name=f"s1_{i}")
                s2_ps = psS.tile([P, LC], F32, tag="sums", name=f"s2_{i}")
                for ec in range(EO):
                    nc.tensor.matmul(s_ps[:], ones_r[:], y_sl[ec],
                                     start=(ec == 0), stop=(ec == EO - 1))
                for ec in range(EO):
                    nc.tensor.matmul(s2_ps[:], ones_r[:], ysq[:, ec, :],
                                     start=(ec == 0), stop=(ec == EO - 1))
                state[("ln1", i)] = ln_stats_rest(i, "1", s_ps, s2_ps, y_sl)

            def ffn_start(i):
                rdt = F8E4 if FFN2_FP8 else F16
                relu1 = prelu.tile([P, FO, LC], rdt, tag="relu1",
                                   name=f"relu1_{i}")
                ao2 = [psF2.tile([P, LC], F32, tag="ao2", name=f"ao2_{i}_{e}")
                       for e in range(EO)]
                state[("ffn", i)] = (relu1, ao2)

            def ffn1(i, fo):
                relu1, _ = state[("ffn", i)]
                h, h8 = state[("h", i)]
                fp = psF1.tile([P, LC], F32, tag="f1", name=f"fp{i}_{fo}")
                if FFN1_FP8:
                    w1s = w1_sl(fo)
                    for pr in range(NPR):
                        nc.tensor.matmul(fp[:], w1s[:, 2 * pr:2 * pr + 2, :],
                                         h8[:, 2 * pr:2 * pr + 2, :],
                                         start=(pr == 0), stop=(pr == NPR - 1),
                                         perf_mode=DR)
                    nc.scalar.activation(relu1[:, fo, :], fp[:], AF.Relu,
                                         bias=b1_t[:, fo:fo + 1],
                                         scale=1.0 / M_SC)
                else:
                    w1s = w1_sl(fo)
                    for kk in range(EO):
                        nc.tensor.matmul(fp[:], w1s[:, kk, :], h[:, kk, :],
                                         start=(kk == 0), stop=(kk == EO - 1))
                    nc.scalar.activation(relu1[:, fo, :], fp[:], AF.Relu,
                                         bias=b1_t[:, fo:fo + 1])

            def ffn2(i, fo, cols=None):
                relu1, ao2 = state[("ffn", i)]
                cl, chi = (0, LC) if cols is None else cols
                if FFN2_FP8:
                    if fo % 2 == 0:
                        return
                    j = fo // 2
                    w2s = (w2e[:, 0:2, :] if j == 0
                           else w2t[:, 2 * j:2 * j + 2, :])
                    for eb in range(EO):
                        nc.tensor.matmul(
                            ao2[eb][:, cl:chi], w2s[:, :, eb * P:(eb + 1) * P],
                            relu1[:, 2 * j:2 * j + 2, cl:chi],
                            start=(j == 0), stop=(j == FO // 2 - 1),
                            perf_mode=DR)
                else:
                    w2s = w2_sl(fo)
                    for eb in range(EO):
                        nc.tensor.matmul(
                            ao2[eb][:, cl:chi], w2s[:, eb * P:(eb + 1) * P],
                            relu1[:, fo, cl:chi],
                            start=(fo == 0), stop=(fo == FO - 1))

            def resid2(i, cols=None, keep=False):
                """z = ffn + h (+b2), squares + partition sums, per-ec
                interleaved so the post-FFN critical path is short."""
                cl, chi = (0, LC) if cols is None else cols
                half = "" if cols is None else f"_{cl}"
                w = chi - cl
                _, ao2 = state[("ffn", i)]
                h, _ = state[("h", i)] if keep else state.pop(("h", i))
                if b2_t is not None:
                    for ec in range(EO):
                        nc.vector.tensor_tensor(
                            h[:, ec, cl:chi], h[:, ec, cl:chi],
                            b2_t[:, ec:ec + 1].to_broadcast((P, w)), OP.add)
                y2 = py2.tile([P, EO, LC], F32R, tag="y2", name=f"y2_{i}")
                ysq = pysq.tile([P, EO, LC], F32R, tag="ysq",
                                name=f"ysq2_{i}{half}")
                s_ps = psS.tile([P, LC], F32, tag="sums", name=f"s3_{i}{half}")
                s2_ps = psS.tile([P, LC], F32, tag="sums", name=f"s4_{i}{half}")
                ff_sc = 1.0 / (M_SC) if FFN2_FP8 else 1.0
                for ec in range(EO):
                    # ao2 is PSUM: GPSIMD can't read it, keep adds on DVE
                    if FFN2_FP8:
                        nc.vector.scalar_tensor_tensor(
                            y2[:, ec, cl:chi], ao2[ec][:, cl:chi], ff_sc,
                            h[:, ec, cl:chi], OP.mult, OP.add)
                    else:
                        nc.vector.tensor_tensor(y2[:, ec, cl:chi],
                                                ao2[ec][:, cl:chi],
                                                h[:, ec, cl:chi], OP.add)
                    nc.scalar.activation(ysq[:, ec, cl:chi],
                                         y2[:, ec, cl:chi].bitcast(F32),
                                         AF.Square)
                    nc.tensor.matmul(s_ps[:, 0:w], ones_r[:], y2[:, ec, cl:chi],
                                     start=(ec == 0), stop=(ec == EO - 1))
                    nc.tensor.matmul(s2_ps[:, 0:w], ones_r[:],
                                     ysq[:, ec, cl:chi],
                                     start=(ec == 0), stop=(ec == EO - 1))
                if not keep:
                    state.pop(("ffn", i))
                state[("y2", i) if cols is None else ("y2", i, cl)] = \
                    (y2, s_ps, s2_ps)

            def ln2_full(i, is_last=False, cols=None):
                cl, chi = (0, LC) if cols is None else cols
                w = chi - cl
                key = ("y2", i) if cols is None else ("y2", i, cl)
                y2, s_ps, s2_ps = state.pop(key)
                y2_sl = [y2[:, ec, cl:chi] for ec in range(EO)]
                _, negmean, rstd = ln_stats_rest(
                    i, "2" if cols is None else f"2_{cl}",
                    s_ps[:, 0:w], s2_ps[:, 0:w], y2_sl)
                ls = i * LC + cl
                outt = pout.tile([P, EO, LC], F32, tag="out",
                                 name=f"out{i}_{cl}")
                # subtracts only need negmean — they hide under the ACT sqrt
                # and DVE reciprocal that produce rstd
                ts = []
                for ec in range(EO):
                    t = pstat.tile([P, LC], F32, tag=f"lnapp{ec}",
                                   name=f"la2_{i}_{ec}_{cl}")
                    nc.vector.tensor_tensor(t[:, 0:w], y2_sl[ec].bitcast(F32),
                                            negmean[:], OP.add)
                    ts.append(t)
                for ec in range(EO):
                    t = ts[ec]
                    if ln2_trivial:
                        nc.vector.tensor_tensor(outt[:, ec, cl:chi], t[:, 0:w],
                                                rstd[:], OP.mult)
                    else:
                        nc.vector.tensor_tensor(t[:, 0:w], t[:, 0:w], rstd[:],
                                                OP.mult)
                        nc.scalar.activation(outt[:, ec, cl:chi], t[:, 0:w],
                                             AF.Identity,
                                             bias=ln2b_t[:, ec:ec + 1],
                                             scale=ln2w_t[:, ec:ec + 1])
                    nc.sync.dma_start(out_r[:, ec, ls:ls + w],
                                      outt[:, ec, cl:chi])

            # ---- pipelined emission (h(0) already computed in attention) ----
            for i in range(NLC):
                last = i == NLC - 1
                h0 = (0, LC // 2)
                h1 = (LC // 2, LC)
                c0 = h0 if last else None
                ffn_start(i)
                ffn1(i, 0)
                ffn1(i, 1)
                if i > 0:
                    ln2_full(i - 1)          # PE: 8 stats MMs amid FFN stream
                ffn2(i, 0, c0)
                ffn1(i, 2)
                ffn2(i, 1, c0)
                ffn1(i, 3)
                ffn2(i, 2, c0)
                if i + 1 < NLC:
                    ln_pre(i + 1)            # next chunk's LN1 stats
                ffn1(i, 4)
                ffn2(i, 3, c0)
                ffn1(i, 5)
                ffn2(i, 4, c0)
                ffn1(i, 6)
                ffn2(i, 5, c0)
                if i + 1 < NLC:
                    ln1_apply(i + 1)         # h(i+1) ready before FFN(i) ends
                for fo in range(7, FO):
                    ffn1(i, fo)
                    ffn2(i, fo - 1, c0)
                ffn2(i, FO - 1, c0)
                if not last:
                    resid2(i)
                else:
                    # column-split tail: half-1 matmuls hide half-0's LN2 chain
                    resid2(i, cols=h0, keep=True)
                    for fo in range(FO):
                        ffn2(i, fo, h1)
                        if fo == 7:
                            ln2_full(i, is_last=True, cols=h0)
                    resid2(i, cols=h1)
            ln2_full(NLC - 1, is_last=True, cols=h1)

    nc.compile()
    return nc


def kernel(x, in_proj_w, ln1_w, ln1_b, ln2_w, ln2_b, w1, b1, w2, b2):
    global LAST_RESULT
    x = np.asarray(x, dtype=np.float32)
    in_proj_w = np.asarray(in_proj_w, dtype=np.float32)
    w1 = np.asarray(w1, dtype=np.float32)
    w2 = np.asarray(w2, dtype=np.float32)
    b1 = np.asarray(b1, dtype=np.float32)
    b2 = np.asarray(b2, dtype=np.float32)
    ln1_w = np.asarray(ln1_w, dtype=np.float32)
    ln1_b = np.asarray(ln1_b, dtype=np.float32)
    ln2_w = np.asarray(ln2_w, dtype=np.float32)
    ln2_b = np.asarray(ln2_b, dtype=np.float32)

    ln1_trivial = bool(np.all(ln1_w == 1.0) and np.all(ln1_b == 0.0))
    ln2_trivial = bool(np.all(ln2_w == 1.0) and np.all(ln2_b == 0.0))
    b2_zero = bool(np.all(b2 == 0.0))

    key = (ln1_trivial, ln2_trivial, b2_zero)
    if key not in _CACHE:
        _CACHE[key] = _build(*key)
    nc = _CACHE[key]

    E4NP = ml_dtypes.float8_e4m3
    wq = in_proj_w[:E].astype(np.float64)
    wk = in_proj_w[E:2 * E].astype(np.float64)
    m8 = (M_SC * SCALE * (wq.T @ wk)).astype(np.float32).astype(E4NP)  # [E, E]
    wv8 = (WV_SC * in_proj_w[2 * E:].T).astype(E4NP)
    w1t = ((M_SC * w1.T).astype(E4NP) if FFN1_FP8
           else w1.T.astype(np.float16))                         # [E, FF]
    w2t = ((M_SC * w2.T).astype(E4NP) if FFN2_FP8
           else w2.T.astype(np.float16))                         # [FF, E]

    in_maps = []
    for bb in range(B):
        xtb = _round_fp32r(x[bb].T)
        m = {
            "xt": xtb,                                           # [E, L]
            "x8": xtb.astype(E4NP),
            "m8": m8, "wv8": wv8,
            "w1t": w1t, "w2t": w2t, "b1v": b1,
        }
        if not b2_zero:
            m["b2v"] = b2
        if not ln1_trivial:
            m["ln1w"] = ln1_w
            m["ln1b"] = ln1_b
        if not ln2_trivial:
            m["ln2w"] = ln2_w
            m["ln2b"] = ln2_b
        in_maps.append(m)

    res = run_bass_kernel_spmd(nc, in_maps, list(range(B)), trace=_TRACE)
    LAST_RESULT = res
    out = np.stack([np.ascontiguousarray(res.results[bb]["outt"].T)
                    for bb in range(B)])
    return out.astype(np.float32)
